# revision 15
# baseline (speedup 1.0000x reference)
"""Trainium2 Bass kernel for nn_CausalNet (block-diagonal GNN + BN + MLP head).

Strategy: data-parallel over batch (8 samples/core on 8 cores).
 - Feature-major layouts; every BN/broadcast is per-partition.
 - Row norms from the Gram diagonal, degrees via per-block [128,1] matmuls,
   reciprocals batched on [128,32] tiles: no gpsimd partition reduces.
 - x/W1/h1/W2/h2/Wm1 in bf16 for matmul FWL + halved traffic.
 - Biases b1/b2 added via broadcast tiles fused into the PSUM->SBUF copy;
   bm1/bm2 dropped (cancel inside BatchNorm1d).
 - BN stats exchanged via AllGather + local DVE reduce (single-step collective
   instead of ring AllReduce); applies split across scalar+DVE engines.
 - Readout: j-sliced AllToAll (each core owns 32 h-features x all patches)
   in bf16 with 1KB-contiguous descriptors; consumer does ONE transposing
   DMA into [128,128,64] chunks and contracts against a pre-permuted bf16
   Wm1 slice; 32KB AllReduce combines partials.
"""
import sys
import numpy as np

sys.path.insert(0, "/opt/trn_rl_repo")

B, N, P, D = 64, 4, 128, 256
H = 256
TOTP = N * P          # 512
NCORES = 8
BLOC = B // NCORES    # 8 samples per core
T = BLOC * TOTP       # 4096 tokens per core
NB = BLOC * N         # 32 (sample, subgraph) blocks per core
JSL = H // NCORES     # 32 h-features per core slice (readout)
NK = 128              # readout chunks of 128 features
JT = H // 128         # 2 feature partition-tiles
EPS_BN = 1e-5
CNT1 = float(B * TOTP)   # BN denominator for GCN layers
CNT2 = float(B)          # BN denominator for head


def build_bass(repeat=1, no_cc=False):
    import concourse.bass as bass
    import concourse.bacc as bacc
    import concourse.mybir as mybir
    import concourse.tile as tile

    f32 = mybir.dt.float32
    bf16 = mybir.dt.bfloat16
    Act = mybir.ActivationFunctionType
    Alu = mybir.AluOpType
    AX = mybir.AxisListType

    nc = bacc.Bacc("TRN2", target_bir_lowering=False, debug=False,
                   num_devices=NCORES)

    def inp(name, shape, dt=f32):
        return nc.dram_tensor(name, shape, dt, kind="ExternalInput")

    xT_d = inp("xT", [D, T], bf16)      # d-major activations for this core
    W1_d = inp("W1b", [D, H], bf16)
    b1_d = inp("b1bc", [128, H])        # b1 broadcast to 128 partitions
    g1_d = inp("g1p", [128, JT])        # column jh = features [jh*128,(jh+1)*128)
    be1_d = inp("be1p", [128, JT])
    W2_d = inp("W2b", [H, H], bf16)
    b2_d = inp("b2bc", [128, H])
    g2_d = inp("g2p", [128, JT])
    be2_d = inp("be2p", [128, JT])
    mAT_d = inp("mAT", [P, 4 * P])      # (0.5*mask*(1-I)).T tiled x4
    mBT_d = inp("mBT", [P, 4 * P])      # (0.5*mask*(1-I) + I).T tiled x4
    idT_d = inp("idT", [P, 4 * P])      # identity tiled x4 (diag mask)
    Wm1_d = inp("Wm1s", [128, NK * 128], bf16)  # permuted slice [p, (k,o)]
    gm1_d = inp("gm1", [128, 1])
    bem1_d = inp("bem1", [128, 1])
    Wm2_d = inp("Wm2", [128, 64])
    gm2_d = inp("gm2", [64, 1])
    bem2_d = inp("bem2", [64, 1])
    Wm3_d = inp("Wm3", [64, 2])
    bm3_d = inp("bm3", [2, 1])
    onec_d = inp("ones_col", [128, 1])
    sel0_d = inp("sel0", [128, JSL])
    sel1_d = inp("sel1", [128, JSL])
    id32_d = inp("id32", [32, 32])

    out_ext = nc.dram_tensor("out", [2, B], f32, kind="ExternalOutput")

    with tile.TileContext(nc) as tc:
        with (
            tc.tile_pool(name="persist", bufs=1) as pp,
            tc.tile_pool(name="work", bufs=3) as wp,
            tc.tile_pool(name="small", bufs=2) as sp,
            tc.tile_pool(name="scratch", bufs=1) as scp,
            tc.tile_pool(name="ps", bufs=3, space="PSUM") as ps,
            tc.tile_pool(name="psx", bufs=2, space="PSUM") as psx,
            tc.tile_pool(name="ps1", bufs=1, space="PSUM") as ps1,
            tc.tile_pool(name="dram", bufs=1, space="DRAM") as dp,
        ):
            # ---------------- persistent SBUF ----------------
            def load(name, dram, shape, sl=None, dt=f32):
                t = pp.tile(shape, dt, tag=name, name=name)
                nc.sync.dma_start(out=t[:], in_=dram[:] if sl is None else sl)
                return t

            xTs = [load(f"xT{k}", xT_d, [128, T], xT_d[k * 128:(k + 1) * 128, :],
                        dt=bf16)
                   for k in range(2)]
            W1s = [load(f"W1{k}", W1_d, [128, H], W1_d[k * 128:(k + 1) * 128, :],
                        dt=bf16)
                   for k in range(2)]
            W2s = [load(f"W2{k}", W2_d, [128, H], W2_d[k * 128:(k + 1) * 128, :],
                        dt=bf16)
                   for k in range(2)]
            b1s = load("b1", b1_d, [128, H])
            b2s = load("b2", b2_d, [128, H])
            mATs = load("mAT", mAT_d, [P, 4 * P])
            mBTs = load("mBT", mBT_d, [P, 4 * P])
            idTs = load("idT", idT_d, [P, 4 * P])
            onec = load("onec", onec_d, [128, 1])
            sel0s = load("sel0", sel0_d, [128, JSL])
            sel1s = load("sel1", sel1_d, [128, JSL])
            id32s = load("id32", id32_d, [32, 32])
            g1s = load("g1", g1_d, [128, JT])
            be1s = load("be1", be1_d, [128, JT])
            g2s = load("g2", g2_d, [128, JT])
            be2s = load("be2", be2_d, [128, JT])
            gm1s = load("gm1", gm1_d, [128, 1])
            bem1s = load("bem1", bem1_d, [128, 1])
            gm2s = load("gm2", gm2_d, [64, 1])
            bem2s = load("bem2", bem2_d, [64, 1])
            Wm2s = load("Wm2", Wm2_d, [128, 64])
            Wm3s = load("Wm3", Wm3_d, [64, 2])
            bm3s = load("bm3", bm3_d, [2, 1])
            Wm1s = load("Wm1s", Wm1_d, [128, NK * 128], dt=bf16)

            epsb = pp.tile([128, 1], f32, tag="epsb")
            nc.vector.memset(epsb[:], EPS_BN)
            AnT = pp.tile([128, NB * P], f32, tag="AnT")     # scaled A^T blocks
            dqA = pp.tile([1, 128 * NB], f32, tag="dqA")
            h1B = [pp.tile([128, T], bf16, tag=f"h1B{k}", name=f"h1B{k}")
                   for k in range(JT)]
            h2B = [pp.tile([128, T], bf16, tag=f"h2B{k}", name=f"h2B{k}")
                   for k in range(JT)]
            tps = pp.tile([128, NK, 64], bf16, tag="tps")    # transposed chunks

            rg = [list(range(NCORES))]

            def cc(kind, op, cin, cout):
                nc.gpsimd.collective_compute(
                    kind, op, replica_groups=rg,
                    ins=[cin.opt()], outs=[cout.opt()])

            for _rep in range(repeat):
                st1_in = dp.tile([128, 8, 4], f32, tag="st1i", name="st1_in")
                st1_out = dp.tile([NCORES, 128, 8, 4], f32, tag="st1o",
                                  addr_space="Shared", name="st1_out")
                st2_in = dp.tile([128, 8, 4], f32, tag="st2i", name="st2_in")
                st2_out = dp.tile([NCORES, 128, 8, 4], f32, tag="st2o",
                                  addr_space="Shared", name="st2_out")
                a2a_in = dp.tile([NCORES, BLOC, JSL, TOTP], bf16, tag="a2ai",
                                 name="a2a_in")
                a2a_out = dp.tile([NCORES, BLOC, JSL, TOTP], bf16, tag="a2ao",
                                  name="a2a_out")
                z1_in = dp.tile([128, 64], f32, tag="z1i", name="z1_in")
                z1_out = dp.tile([NCORES, 128, 64], f32, tag="z1o",
                                 addr_space="Shared", name="z1_out")

                SB = NB // 4
                # ---- pass A: Gram blocks + norms from the diagonal ----
                Gss, rqs = [], []
                for sb in range(SB):
                    c0 = sb * 4 * P
                    G4 = ps.tile([P, 4 * P], f32, tag="big")
                    for b in range(4):
                        cb = c0 + b * P
                        for kt in range(2):
                            nc.tensor.matmul(
                                G4[:, b * P:(b + 1) * P],
                                xTs[kt][:, cb:cb + P], xTs[kt][:, cb:cb + P],
                                start=(kt == 0), stop=(kt == 1),
                            )
                    Gs = wp.tile([P, 4 * P], f32, tag="Gs", bufs=8)
                    nc.vector.tensor_copy(Gs[:], G4[:])
                    Gss.append(Gs)
                    dsc = scp.tile([P, 4 * P], f32, tag="dsc")
                    nc.vector.tensor_mul(dsc[:], G4[:], idTs[:])
                    nsq = sp.tile([128, 4], f32, tag="nsq")
                    nc.vector.reduce_sum(
                        nsq[:], dsc[:].rearrange("p (b q) -> p b q", b=4), AX.X)
                    nrm = sp.tile([128, 4], f32, tag="nrm")
                    nc.scalar.activation(nrm[:], nsq[:], Act.Sqrt)
                    rcol = sp.tile([128, 4], f32, tag="rcol", bufs=8)
                    nc.vector.reciprocal(rcol[:], nrm[:])
                    rq = sp.tile([1, 512], f32, tag="rq", bufs=8)
                    nc.sync.dma_start(
                        out=rq[:].rearrange("o (p b) -> o p b", b=4),
                        in_=rcol[:])
                    rqs.append(rq)

                # ---- pass B: A^T blocks + degree columns ----
                degp = ps1.tile([128, NB], f32, tag="acc")
                for sb in range(SB):
                    c0 = sb * 4 * P
                    R4 = ps.tile([P, 4 * P], f32, tag="big")
                    rqv = rqs[sb][:].rearrange("o (p b) -> o p b", b=4)
                    for b in range(4):
                        nc.tensor.matmul(R4[:, b * P:(b + 1) * P],
                                         rqv[:, :, b], rqv[:, :, b],
                                         start=True, stop=True)
                    AT = AnT[:, c0:c0 + 4 * P]
                    nc.vector.tensor_mul(AT, Gss[sb][:], mATs[:])
                    nc.vector.tensor_mul(AT, AT, R4[:])
                    nc.vector.tensor_add(AT, AT, mBTs[:])
                    for b in range(4):
                        i = sb * 4 + b
                        nc.tensor.matmul(degp[:, i:i + 1],
                                         AnT[:, c0 + b * P:c0 + (b + 1) * P],
                                         onec[:], start=True, stop=True)

                # ---- dinv rows, fold D^-1/2 into AnT ----
                dnr = sp.tile([128, NB], f32, tag="dnr")
                nc.scalar.activation(dnr[:], degp[:], Act.Sqrt)
                dcol = sp.tile([128, NB], f32, tag="dcol")
                nc.vector.reciprocal(dcol[:], dnr[:])
                nc.sync.dma_start(
                    out=dqA[:].rearrange("o (p i) -> o p i", i=NB),
                    in_=dcol[:])
                dqv = dqA[:].rearrange("o (p i) -> o p i", i=NB)
                for sb in range(SB):
                    c0 = sb * 4 * P
                    Do4 = ps.tile([P, 4 * P], f32, tag="big")
                    for b in range(4):
                        i = sb * 4 + b
                        nc.tensor.matmul(Do4[:, b * P:(b + 1) * P],
                                         dqv[:, :, i], dqv[:, :, i],
                                         start=True, stop=True)
                    nc.vector.tensor_mul(AnT[:, c0:c0 + 4 * P],
                                         AnT[:, c0:c0 + 4 * P], Do4[:])

                # ---- layer 1: xw = x@W1 (+b1 fused) ; h1 = An @ xw ----
                stts1 = []
                for sb in range(SB):
                    c0 = sb * 4 * P
                    xws = []
                    for b in range(4):
                        cb = c0 + b * P
                        xw_ps = psx.tile([128, H], f32, tag="xw")
                        for kt in range(2):
                            nc.tensor.matmul(
                                xw_ps[:], xTs[kt][:, cb:cb + P], W1s[kt][:],
                                start=(kt == 0), stop=(kt == 1),
                            )
                        xw = wp.tile([128, H], f32, tag="xw", bufs=5)
                        nc.vector.tensor_add(xw[:], xw_ps[:], b1s[:])
                        xws.append(xw)
                    for jh in range(JT):
                        hh4 = ps.tile([128, 4 * P], f32, tag="big")
                        for b in range(4):
                            cb = c0 + b * P
                            nc.tensor.matmul(
                                hh4[:, b * P:(b + 1) * P],
                                xws[b][:, jh * 128:(jh + 1) * 128],
                                AnT[:, cb:cb + P],
                                start=True, stop=True,
                            )
                        nc.scalar.activation(h1B[jh][:, c0:c0 + 4 * P], hh4[:],
                                             Act.Copy)
                        if jh == 0:
                            stt = sp.tile([128, 4], f32, tag="stt", bufs=8)
                            stts1.append(stt)
                        stt = stts1[sb]
                        nc.vector.reduce_sum(stt[:, jh:jh + 1], hh4[:], AX.X)
                        sqs = wp.tile([128, 4 * P], bf16, tag="sqs", bufs=3)
                        nc.scalar.activation(sqs[:], hh4[:], Act.Square,
                                             accum_out=stt[:, 2 + jh:3 + jh])
                    nc.gpsimd.dma_start(out=st1_in[:, sb, :], in_=stts1[sb][:])

                # ---- BN stats + AllGather + local reduce + split apply ----
                def bn_coeffs(stin, stout, gs, bes):
                    cc("AllGather", Alu.bypass, stin, stout)
                    stg8 = sp.tile([128, NCORES, 8, 4], f32, tag="stg8")
                    nc.sync.dma_start(out=stg8[:],
                                      in_=stout[:].rearrange("r p s c -> p r s c"))
                    stg = sp.tile([128, 4], f32, tag="stg")
                    nc.vector.reduce_sum(
                        stg[:], stg8[:].rearrange("p r s c -> p c r s"), AX.XY)
                    mean = sp.tile([128, 2], f32, tag="mean")
                    nc.vector.tensor_scalar_mul(mean[:], stg[:, 0:2], 1.0 / CNT1)
                    var = sp.tile([128, 2], f32, tag="var")
                    nc.vector.tensor_scalar_mul(var[:], stg[:, 2:4], 1.0 / CNT1)
                    msq = sp.tile([128, 2], f32, tag="msq")
                    nc.vector.tensor_mul(msq[:], mean[:], mean[:])
                    nc.vector.tensor_sub(var[:], var[:], msq[:])
                    sd = sp.tile([128, 2], f32, tag="sd")
                    nc.scalar.activation(sd[:], var[:], Act.Sqrt, bias=epsb[:])
                    rsd = sp.tile([128, 2], f32, tag="rsd")
                    nc.vector.reciprocal(rsd[:], sd[:])
                    a = sp.tile([128, 2], f32, tag="a")
                    nc.vector.tensor_mul(a[:], gs[:], rsd[:])
                    c = sp.tile([128, 2], f32, tag="c")
                    nc.vector.tensor_mul(c[:], mean[:], a[:])
                    nc.vector.tensor_sub(c[:], bes[:], c[:])
                    return a, c

                a1, c1 = bn_coeffs(st1_in, st1_out, g1s, be1s)
                # jh0 on scalar engine, jh1 on DVE (parallel)
                nc.scalar.activation(h1B[0][:], h1B[0][:], Act.Relu,
                                     bias=c1[:, 0:1], scale=a1[:, 0:1])
                nc.vector.tensor_scalar(h1B[1][:], h1B[1][:], a1[:, 1:2],
                                        c1[:, 1:2], Alu.mult, Alu.add)
                nc.vector.tensor_scalar_max(h1B[1][:], h1B[1][:], 0.0)

                # ---- layer 2 (bf16 matmuls) ----
                stts2 = []
                for sb in range(SB):
                    c0 = sb * 4 * P
                    xws = []
                    for b in range(4):
                        cb = c0 + b * P
                        xw_ps = psx.tile([128, H], f32, tag="xw")
                        for jh in range(JT):
                            nc.tensor.matmul(
                                xw_ps[:], h1B[jh][:, cb:cb + P], W2s[jh][:],
                                start=(jh == 0), stop=(jh == 1),
                            )
                        xw = wp.tile([128, H], f32, tag="xw", bufs=5)
                        nc.vector.tensor_add(xw[:], xw_ps[:], b2s[:])
                        xws.append(xw)
                    for jh in range(JT):
                        hh4 = ps.tile([128, 4 * P], f32, tag="big")
                        for b in range(4):
                            cb = c0 + b * P
                            nc.tensor.matmul(
                                hh4[:, b * P:(b + 1) * P],
                                xws[b][:, jh * 128:(jh + 1) * 128],
                                AnT[:, cb:cb + P],
                                start=True, stop=True,
                            )
                        nc.scalar.activation(h2B[jh][:, c0:c0 + 4 * P], hh4[:],
                                             Act.Copy)
                        if jh == 0:
                            stt = sp.tile([128, 4], f32, tag="stt2", bufs=8)
                            stts2.append(stt)
                        stt = stts2[sb]
                        nc.vector.reduce_sum(stt[:, jh:jh + 1], hh4[:], AX.X)
                        sqs = wp.tile([128, 4 * P], bf16, tag="sqs", bufs=3)
                        nc.scalar.activation(sqs[:], hh4[:], Act.Square,
                                             accum_out=stt[:, 2 + jh:3 + jh])
                    nc.gpsimd.dma_start(out=st2_in[:, sb, :], in_=stts2[sb][:])

                # ---- j-sliced AllToAll export of RAW h2 (bf16) ----
                # slot cd holds [s, jp, t] : this core's 8 samples x 32 h x 512 p
                for cd in range(NCORES):
                    jh = cd // 4
                    j0 = (cd % 4) * JSL
                    src = h2B[jh][j0:j0 + JSL, :]
                    dst = a2a_in[cd].rearrange("s j t -> j s t")
                    nc.sync.dma_start(
                        out=dst, in_=src.rearrange("j (s t) -> j s t", s=BLOC))

                a2, c2 = bn_coeffs(st2_in, st2_out, g2s, be2s)
                # select this core's 32 coeffs into broadcast rows
                acs = sp.tile([128, 4], f32, tag="acs")
                nc.vector.tensor_copy(acs[:, 0:2], a2[:])
                nc.vector.tensor_copy(acs[:, 2:4], c2[:])
                acv = acs[:].rearrange("p (g t) -> p t g", t=2)
                selp = psx.tile([32, 2], f32, tag="xw")
                nc.tensor.matmul(selp[:], sel0s[:], acv[:, 0, :],
                                 start=True, stop=False)
                nc.tensor.matmul(selp[:], sel1s[:], acv[:, 1, :],
                                 start=False, stop=True)
                sels = sp.tile([32, 2], f32, tag="sels")
                nc.vector.tensor_copy(sels[:], selp[:])
                abc = sp.tile([128, JSL], f32, tag="abc")
                cbc = sp.tile([128, JSL], f32, tag="cbc")
                for col, dstb in ((0, abc), (1, cbc)):
                    tr = psx.tile([1, 32], f32, tag="xw")
                    nc.tensor.transpose(tr[:], sels[:, col:col + 1], id32s[:])
                    rr = sp.tile([1, 32], f32, tag=f"rr{col}")
                    nc.vector.tensor_copy(rr[:], tr[:])
                    nc.gpsimd.partition_broadcast(dstb[:], rr[:])

                cc("AllToAll", Alu.bypass, a2a_in, a2a_out)

                # ---- readout: one transposing DMA, fused BN, contract Wm1 ----
                a2av = a2a_out[:].rearrange("r s j t -> (r s) (j t)")
                nc.sync.dma_start(out=tps[:, 0:NK // 2, :],
                                  in_=a2av[:, 0:NK * 64], transpose=True)
                nc.sync.dma_start(out=tps[:, NK // 2:NK, :],
                                  in_=a2av[:, NK * 64:NK * 128], transpose=True)
                z1p = ps1.tile([128, 64], f32, tag="acc")
                for jp in range(JSL):
                    grp = tps[:, 4 * jp:4 * (jp + 1), :]
                    if jp % 2 == 0:
                        nc.scalar.activation(grp, grp, Act.Relu,
                                             bias=cbc[:, jp:jp + 1],
                                             scale=abc[:, jp:jp + 1])
                    else:
                        nc.vector.tensor_scalar(grp, grp, abc[:, jp:jp + 1],
                                                cbc[:, jp:jp + 1],
                                                Alu.mult, Alu.add)
                        nc.vector.tensor_scalar_max(grp, grp, 0.0)
                for k in range(NK):
                    nc.tensor.matmul(z1p[:], Wm1s[:, k * 128:(k + 1) * 128],
                                     tps[:, k, :], start=(k == 0),
                                     stop=(k == NK - 1))
                z1s = sp.tile([128, 64], f32, tag="z1s")
                nc.vector.tensor_copy(z1s[:], z1p[:])
                nc.sync.dma_start(out=z1_in[:], in_=z1s[:])
                cc("AllGather", Alu.bypass, z1_in, z1_out)
                z1g8 = sp.tile([128, NCORES, 64], f32, tag="z1g8")
                nc.sync.dma_start(out=z1g8[:],
                                  in_=z1_out[:].rearrange("r p c -> p r c"))
                z1t = sp.tile([128, 64], f32, tag="z1t")
                nc.vector.reduce_sum(
                    z1t[:], z1g8[:].rearrange("p r c -> p c r"), AX.X)

                # ---- head BN + relu (z1t is [feature, sample]) ----
                def head_bn(zt, parts, gs, bes):
                    stm = sp.tile([parts, 1], f32, tag="hstm")
                    nc.vector.reduce_sum(stm[:], zt[:], AX.X)
                    mean = sp.tile([parts, 1], f32, tag="hmean")
                    nc.vector.tensor_scalar_mul(mean[:], stm[:], 1.0 / CNT2)
                    sqs2 = sp.tile([parts, 64], f32, tag="hsq")
                    sts = sp.tile([parts, 1], f32, tag="hsts")
                    nc.scalar.activation(sqs2[:], zt[:], Act.Square,
                                         accum_out=sts[:])
                    var = sp.tile([parts, 1], f32, tag="hvar")
                    nc.vector.tensor_scalar_mul(var[:], sts[:], 1.0 / CNT2)
                    msq = sp.tile([parts, 1], f32, tag="hmsq")
                    nc.vector.tensor_mul(msq[:], mean[:], mean[:])
                    nc.vector.tensor_sub(var[:], var[:], msq[:])
                    sd = sp.tile([parts, 1], f32, tag="hsd")
                    nc.scalar.activation(sd[:], var[:], Act.Sqrt,
                                         bias=epsb[:var.shape[0], :])
                    rsd = sp.tile([parts, 1], f32, tag="hrsd")
                    nc.vector.reciprocal(rsd[:], sd[:])
                    a = sp.tile([parts, 1], f32, tag="ha")
                    nc.vector.tensor_mul(a[:], gs[:], rsd[:])
                    c = sp.tile([parts, 1], f32, tag="hc")
                    nc.vector.tensor_mul(c[:], mean[:], a[:])
                    nc.vector.tensor_sub(c[:], bes[:], c[:])
                    nc.scalar.activation(zt[:], zt[:], Act.Relu, bias=c[:],
                                         scale=a[:])

                head_bn(z1t, 128, gm1s, bem1s)

                z2_ps = psx.tile([64, 64], f32, tag="xw")
                nc.tensor.matmul(z2_ps[:], Wm2s[:], z1t[:], start=True, stop=True)
                z2t = sp.tile([64, 64], f32, tag="z2t")
                nc.vector.tensor_copy(z2t[:], z2_ps[:])
                head_bn(z2t, 64, gm2s, bem2s)

                z3_ps = psx.tile([2, 64], f32, tag="xw")
                nc.tensor.matmul(z3_ps[:], Wm3s[:], z2t[:], start=True, stop=True)
                z3 = sp.tile([2, 64], f32, tag="z3")
                nc.vector.tensor_scalar_add(z3[:], z3_ps[:], bm3s[:])
                nc.sync.dma_start(out=out_ext[:], in_=z3[:])

    nc.finalize()
    return nc


_CACHE = {}


def prepare_in_maps(inputs):
    import ml_dtypes
    bf = ml_dtypes.bfloat16

    x = np.asarray(inputs["x"], np.float32)
    mask = np.asarray(inputs["edge_prior_mask"], np.float32)
    Wm1 = np.asarray(inputs["Wm1"], np.float32)

    mA = 0.5 * mask * (1.0 - np.eye(P, dtype=np.float32))
    mB = mA + np.eye(P, dtype=np.float32)

    def c2(v, parts):  # [2*parts] -> [parts, 2] column-per-tile packing
        return np.ascontiguousarray(
            np.asarray(v, np.float32).reshape(2, parts).T)

    common = {
        "W1b": np.asarray(inputs["W1"], np.float32).astype(bf),
        "b1bc": np.ascontiguousarray(np.tile(
            np.asarray(inputs["b1"], np.float32).reshape(1, H), (128, 1))),
        "g1p": c2(inputs["g1"], 128), "be1p": c2(inputs["be1"], 128),
        "W2b": np.asarray(inputs["W2"], np.float32).astype(bf),
        "b2bc": np.ascontiguousarray(np.tile(
            np.asarray(inputs["b2"], np.float32).reshape(1, H), (128, 1))),
        "g2p": c2(inputs["g2"], 128), "be2p": c2(inputs["be2"], 128),
        "mAT": np.ascontiguousarray(np.tile(mA.T, (1, 4))),
        "mBT": np.ascontiguousarray(np.tile(mB.T, (1, 4))),
        "idT": np.ascontiguousarray(
            np.tile(np.eye(P, dtype=np.float32), (1, 4))),
        "gm1": np.asarray(inputs["gm1"], np.float32).reshape(128, 1),
        "bem1": np.asarray(inputs["bem1"], np.float32).reshape(128, 1),
        "Wm2": np.asarray(inputs["Wm2"], np.float32),
        "gm2": np.asarray(inputs["gm2"], np.float32).reshape(64, 1),
        "bem2": np.asarray(inputs["bem2"], np.float32).reshape(64, 1),
        "Wm3": np.asarray(inputs["Wm3"], np.float32),
        "bm3": np.asarray(inputs["bm3"], np.float32).reshape(2, 1),
        "ones_col": np.ones((128, 1), np.float32),
        "id32": np.eye(32, dtype=np.float32),
    }
    # Wm1 permuted per core: lhsT[p, k*128+o] = Wm1[t*256 + j, o]
    # with t = (k%4)*128 + p, j = c*32 + k//4
    W3 = Wm1.reshape(TOTP, H, 128)            # [t, j, o]
    in_maps = []
    for c in range(NCORES):
        xc = x[c * BLOC:(c + 1) * BLOC].reshape(T, D)
        m = dict(common)
        m["xT"] = np.ascontiguousarray(xc.T).astype(bf)
        sel = np.zeros((128, JSL), np.float32)
        for mm in range(JSL):
            sel[(c % 4) * JSL + mm, mm] = 1.0
        m["sel0"] = sel if c < 4 else np.zeros((128, JSL), np.float32)
        m["sel1"] = sel if c >= 4 else np.zeros((128, JSL), np.float32)
        sub = W3[:, c * JSL:(c + 1) * JSL, :]         # [512, 32, 128]
        sub = sub.reshape(4, 128, JSL, 128)           # [tc, p, jp, o]
        arr = sub.transpose(1, 2, 0, 3)               # [p, jp, tc, o]
        m["Wm1s"] = np.ascontiguousarray(
            arr.reshape(128, NK * 128)).astype(bf)
        in_maps.append(m)
    return in_maps


def kernel(**inputs):
    import concourse.bass_utils as bass_utils

    in_maps = prepare_in_maps(inputs)
    if "nc" not in _CACHE:
        _CACHE["nc"] = build_bass()
    res = bass_utils.run_bass_kernel_spmd(
        _CACHE["nc"], in_maps, core_ids=list(range(NCORES)))
    _CACHE["last"] = res
    out = res.results[0]["out"]  # [2, B]
    return np.ascontiguousarray(np.asarray(out).T)
